# revision 15
# baseline (speedup 1.0000x reference)
"""Enformer dot-product self-attention with central-mask relative position
bias, on 8 Trainium2 NeuronCores (one head per core, SPMD).

Math per head h (S=2048, D=64, N=64):
    basis[i,j,:] = f(d=i-j)  — indicator features, zero for |d| > 1024
    logits = (q @ k^T + (q @ w) @ basis^T + u @ k^T + (v @ w) @ basis^T) / 8
    out    = softmax(logits) @ value

Device formulation per core (v3 — transposed/j-major attention):
  - qTb [65,S] bf16: rows 0..63 = (q/8)^T, row 64 = ones.
  - kaug [65,S] bf16: rows 0..63 = k^T, row 64 = u@k^T/8 (host-computed).
    Logits are built TRANSPOSED: Lt[j,i] = kaug_tile^T @ qTb  (kaug j-tile
    stationary, qTb streaming), so softmax rows live on the free axis and
    attn@v needs no P transposes at all.
  - Band term in c'-space: T[i,c'] = qTb[:,i].w2r[:,c'], c' = j-i+1024,
    computed per i-tile (A phase), written bf16 to one DRAM tensor G with
    row pitch Q=2176 (2049 band cols + 127 zero pad).
  - The shifted+TRANSPOSED band btT[jp,i] for each j-tile comes back in a
    single xbar DMA: source AP [[Q-1, iw],[1,128]] reads G along the
    diagonal; transpose=True lands it j-major.  Out-of-band reads hit the
    zero pad of the neighbouring row.
  - Band add: PE accumulation (identity @ btT into the qk PSUM group) for
    even j-tiles, DVE tensor add for odd ones — balances both engines.
  - P = exp(Lt) via ScalarE, bf16, straight into sb_Pt[j-tile][i].
  - attn@v per 512-i-chunk: po[65,512] += vba[j-tile]^T @ Pt, accumulated
    over the 16 j-tiles in PSUM; vba has a ones column so row 64 is the
    softmax denominator.  Host does the final transpose + normalize.
"""

import numpy as np
import ml_dtypes

import concourse.bass as bass
import concourse.bacc as bacc
import concourse.mybir as mybir
import concourse.tile as tile
from concourse.bass_utils import run_bass_kernel_spmd
from concourse.masks import make_identity

S = 2048
D = 64
NB = 64          # pos-emb dim (basis features)
H = 8
HALF = NB // 2   # 32
BAND = 1024      # max |d| with nonzero features
Q = S + 128      # G row pitch (2049 band cols + 127 zero pad)
NT = S // 128    # 16 i/j tiles
F32 = mybir.dt.float32
BF16 = mybir.dt.bfloat16

_NC_CACHE = {}


def _basis_feature_matrix():
    """Rr[c', n] for c' in [0, Q): features of distance d = 1024 - c'.
    Matches reference._relative_basis numerics (float32)."""
    pow_rate = np.float32(np.exp(np.log((S + 1) / 2) / HALF))
    widths = np.power(pow_rate, np.arange(1, HALF + 1, dtype=np.float32),
                      dtype=np.float32)  # [32]
    d = (np.float32(BAND) - np.arange(Q, dtype=np.float32))[:, None]  # [Q,1]
    unsigned = (np.abs(d) <= widths[None, :]).astype(np.float32)      # [Q,32]
    signed = np.sign(d) * unsigned
    return np.concatenate([unsigned, signed], axis=-1)  # [Q, 64]


def _jwin(jb):
    j0 = jb * 128
    ilo = max(0, j0 - BAND)
    ihi = min(S, j0 + 128 + BAND)
    return j0, ilo, ihi


def _build_nc():
    if "nc" in _NC_CACHE:
        return _NC_CACHE["nc"]

    nc = bacc.Bacc("TRN2", target_bir_lowering=False, debug=False,
                   num_devices=H)
    d_q = nc.dram_tensor("qTb", [65, S], BF16, kind="ExternalInput")
    d_k = nc.dram_tensor("kaug", [65, S], BF16, kind="ExternalInput")
    d_w2r = nc.dram_tensor("w2r", [65, Q], BF16, kind="ExternalInput")
    d_vba = nc.dram_tensor("vba", [128 * NT * 65], BF16, kind="ExternalInput")
    d_oT = nc.dram_tensor("oT", [65, S], F32, kind="ExternalOutput")
    d_G = nc.dram_tensor("gband", [S * Q], BF16, kind="Internal")

    with tile.TileContext(nc) as tc:
        with tc.tile_pool(name="pers", bufs=1) as pers, \
             tc.tile_pool(name="gsb", bufs=3) as gsb, \
             tc.tile_pool(name="ps_wk", bufs=3, space="PSUM") as ps_wk, \
             tc.tile_pool(name="ps_av", bufs=2, space="PSUM") as ps_av:

            # ---- persistent SBUF ----
            sb_w2r = pers.tile([65, Q], BF16)
            # chunks ordered so A(0)'s window [897, 2049) lands first
            for c in (1, 2, 3, 0):
                lo, hi = c * 544, min(Q, (c + 1) * 544)
                nc.gpsimd.dma_start(out=sb_w2r[:, lo:hi], in_=d_w2r[:, lo:hi])
            sb_q = pers.tile([65, S], BF16)
            nc.gpsimd.dma_start(out=sb_q[:, 0:128], in_=d_q[:, 0:128])
            nc.gpsimd.dma_start(out=sb_q[:, 128:1024], in_=d_q[:, 128:1024])
            nc.gpsimd.dma_start(out=sb_q[:, 1024:S], in_=d_q[:, 1024:S])
            sb_k = pers.tile([65, S], BF16)
            for c in range(2):
                nc.gpsimd.dma_start(out=sb_k[:, c * 1024:(c + 1) * 1024],
                                    in_=d_k[:, c * 1024:(c + 1) * 1024])
            sb_vb = pers.tile([128, NT, 65], BF16)
            rdv = bass.AP(tensor=d_vba, offset=0,
                          ap=[[NT * 65, 128], [1, NT * 65]])
            nc.gpsimd.dma_start(out=sb_vb[:], in_=rdv)
            sb_idb = pers.tile([128, 128], BF16)
            make_identity(nc, sb_idb[:])
            sb_Pt = pers.tile([128, NT, S], BF16)   # [jp, jb, i]
            sb_bt = pers.tile([128, NT, Q], BF16)   # [jp, jb, i - ilo]
            sb_oT = pers.tile([65, S], F32)

            def phase_A(t, eflip=[0]):
                """Band tile t in c'-space -> G rows [t*128, t*128+128)."""
                i0 = t * 128
                jlo = max(0, i0 - BAND)
                jhi = min(S, i0 + 128 + BAND)
                clo = max(0, (jlo - i0 + BAND) - 127)
                chi = min(2049, (jhi - 1) - i0 + BAND + 1)
                gt = gsb.tile([128, Q], BF16)
                nc.gpsimd.memset(gt[:, chi:Q], 0.0)
                cuts = list(range(clo, chi, 1024)) + [chi]
                for ci in range(len(cuts) - 1):
                    lo, hi = cuts[ci], cuts[ci + 1]
                    pg = ps_wk.tile([128, 1024], F32, tag="wk")
                    for s in range(0, hi - lo, 512):
                        se = min(hi - lo, s + 512)
                        nc.tensor.matmul(
                            pg[:, s:se],
                            lhsT=sb_q[:, i0:i0 + 128],
                            rhs=sb_w2r[:, lo + s:lo + se],
                            start=True, stop=True)
                    if eflip[0] % 2 == 0:
                        nc.vector.tensor_copy(gt[:, lo:hi], pg[:, 0:hi - lo])
                    else:
                        nc.scalar.copy(out=gt[:, lo:hi], in_=pg[:, 0:hi - lo])
                    eflip[0] += 1
                wr = bass.AP(tensor=d_G, offset=i0 * Q + clo,
                             ap=[[Q, 128], [1, Q - clo]])
                nc.gpsimd.dma_start(out=wr, in_=gt[:, clo:Q])

            def read_B(jb, h):
                """Shifted+transposed band (i-half h) for j-tile jb."""
                j0, ilo, ihi = _jwin(jb)
                rlo = max(ilo, h * 1024)
                rhi = min(ihi, (h + 1) * 1024)
                if rlo >= rhi:
                    return
                rd = bass.AP(tensor=d_G, offset=rlo * (Q - 1) + j0 + BAND,
                             ap=[[Q - 1, rhi - rlo], [1, 128]])
                nc.sync.dma_start(
                    out=sb_bt[:, jb, rlo - ilo:rhi - ilo], in_=rd,
                    transpose=True)

            def phase_C(jb, h):
                """Transposed logits + band + exp for j-tile jb, i-half h."""
                j0, ilo, ihi = _jwin(jb)
                pq = ps_wk.tile([128, 1024], F32, tag="wk")
                alo = max(ilo, h * 1024)
                ahi = min(ihi, (h + 1) * 1024)
                spans = []
                for c in range(2):
                    i0 = h * 1024 + c * 512
                    # in-band sub-range of this 512-chunk
                    blo = max(alo, i0)
                    bhi = min(ahi, i0 + 512)
                    on_pe = (jb % 2 == 0) and blo < bhi
                    spans.append((blo, bhi) if on_pe else None)
                    nc.tensor.matmul(
                        pq[:, c * 512:(c + 1) * 512],
                        lhsT=sb_k[:, j0:j0 + 128],
                        rhs=sb_q[:, i0:i0 + 512],
                        start=True, stop=not on_pe)
                for sp in spans:
                    if sp is not None:
                        blo, bhi = sp
                        nc.tensor.matmul(
                            pq[:, blo - h * 1024:bhi - h * 1024],
                            lhsT=sb_idb[:],
                            rhs=sb_bt[:, jb, blo - ilo:bhi - ilo],
                            start=False, stop=True)
                if jb % 2 == 1 and alo < ahi:
                    nc.vector.tensor_tensor(
                        pq[:, alo - h * 1024:ahi - h * 1024],
                        pq[:, alo - h * 1024:ahi - h * 1024],
                        sb_bt[:, jb, alo - ilo:ahi - ilo],
                        op=mybir.AluOpType.add)
                nc.scalar.activation(
                    out=sb_Pt[:, jb, h * 1024:(h + 1) * 1024], in_=pq[:],
                    func=mybir.ActivationFunctionType.Exp)

            def phase_AV(c):
                po = ps_av.tile([65, 512], F32, tag="po")
                for jb in range(NT):
                    nc.tensor.matmul(
                        po[:],
                        lhsT=sb_vb[:, jb, :],
                        rhs=sb_Pt[:, jb, c * 512:(c + 1) * 512],
                        start=(jb == 0), stop=(jb == NT - 1))
                nc.vector.tensor_copy(sb_oT[:, c * 512:(c + 1) * 512], po[:])

            # ---- schedule ----
            # h=0 reads only need A(0..7); emit A descending so the h=0
            # sweep (descending jb) unlocks tile by tile.  A(8..15) and the
            # h=1 reads interleave with the h=0 sweep.
            for t in range(7, -1, -1):
                phase_A(t)
            for jb in range(NT - 1, -1, -1):
                read_B(jb, 0)
            for x, jb in enumerate(range(NT - 1, -1, -1)):
                phase_C(jb, 0)
                if x % 2 == 0 and x // 2 < 8:
                    phase_A(8 + x // 2)
                else:
                    read_B(x // 2, 1)
            for jb in range(8, NT):
                read_B(jb, 1)
            phase_AV(0)
            phase_AV(1)
            for jb in range(NT):
                phase_C(jb, 1)
            phase_AV(2)
            phase_AV(3)
            for c in range(4):
                nc.sync.dma_start(out=d_oT[:, c * 512:(c + 1) * 512],
                                  in_=sb_oT[:, c * 512:(c + 1) * 512])

    nc.finalize()
    _NC_CACHE["nc"] = nc
    return nc


def _host_prep(query, key, value, u, v, w):
    """Build the 8 per-core input maps from the full inputs."""
    q = np.asarray(query, np.float32)[0]   # [S,H,D]
    k = np.asarray(key, np.float32)[0]
    val = np.asarray(value, np.float32)[0]
    u = np.asarray(u, np.float32)
    v = np.asarray(v, np.float32)
    w = np.asarray(w, np.float32)
    Rr = _basis_feature_matrix()           # [Q, 64]

    in_maps = []
    for h in range(H):
        q8 = q[:, h, :] / np.float32(8.0)                  # [S,64]
        vw8 = (v[h] @ w[h]) / np.float32(8.0)              # [64]
        qTb = np.concatenate([q8.T, np.ones((1, S), np.float32)], axis=0)
        uk8 = (u[h] / np.float32(8.0)) @ k[:, h, :].T      # [S]
        kaug = np.concatenate([k[:, h, :].T, uk8[None]], axis=0)  # [65,S]
        w2r = np.concatenate([w[h] @ Rr.T, (vw8 @ Rr.T)[None]], axis=0)
        vba = np.concatenate([val[:, h, :], np.ones((S, 1), np.float32)],
                             axis=1)                       # [S,65]
        # device layout [128, NT*65]: row p holds [v|1] for s = jb*128+p
        vba_dev = vba.reshape(NT, 128, 65).transpose(1, 0, 2).reshape(128, -1)
        in_maps.append({
            "qTb": np.ascontiguousarray(qTb).astype(ml_dtypes.bfloat16),
            "kaug": np.ascontiguousarray(kaug).astype(ml_dtypes.bfloat16),
            "w2r": np.ascontiguousarray(w2r).astype(ml_dtypes.bfloat16),
            "vba": np.ascontiguousarray(vba_dev).astype(
                ml_dtypes.bfloat16).reshape(-1),
        })
    return in_maps


def kernel(query, key, value, u, v, w, _trace=False):
    nc = _build_nc()
    in_maps = _host_prep(query, key, value, u, v, w)
    res = run_bass_kernel_spmd(nc, in_maps, core_ids=list(range(H)),
                               trace=_trace)
    outs = []
    for h in range(H):
        oT = res.results[h]["oT"]                # [65, S]
        outs.append((oT[0:D] / oT[D:D + 1]).T)   # [S, D]
    full = np.stack(outs, axis=1)[None]          # [1,S,H,D]
    out = np.ascontiguousarray(full.astype(np.float32))
    if _trace:
        return out, res
    return out


if __name__ == "__main__":
    rng = np.random.default_rng(0)
    ins = {
        "query": rng.standard_normal((1, S, H, D), np.float32),
        "key": rng.standard_normal((1, S, H, D), np.float32),
        "value": rng.standard_normal((1, S, H, D), np.float32),
        "u": rng.standard_normal((H, D), np.float32),
        "v": rng.standard_normal((H, D), np.float32),
        "w": rng.standard_normal((H, D, NB), np.float32),
    }
    out = kernel(**ins)
    print("out shape:", out.shape, "finite:", np.isfinite(out).all())


# revision 19
# speedup vs baseline: 1.0047x; 1.0047x over previous
"""Enformer dot-product self-attention with central-mask relative position
bias, on 8 Trainium2 NeuronCores (one head per core, SPMD).

Math per head h (S=2048, D=64, N=64):
    basis[i,j,:] = f(d=i-j)  — indicator features, zero for |d| > 1024
    logits = (q @ k^T + (q @ w) @ basis^T + u @ k^T + (v @ w) @ basis^T) / 8
    out    = softmax(logits) @ value

Device formulation per core (v3 — transposed/j-major attention):
  - qTb [65,S] bf16: rows 0..63 = (q/8)^T, row 64 = ones.
  - kaug [65,S] bf16: rows 0..63 = k^T, row 64 = u@k^T/8 (host-computed).
    Logits are built TRANSPOSED: Lt[j,i] = kaug_tile^T @ qTb  (kaug j-tile
    stationary, qTb streaming), so softmax rows live on the free axis and
    attn@v needs no P transposes at all.
  - Band term in c'-space: T[i,c'] = qTb[:,i].w2r[:,c'], c' = j-i+1024,
    computed per i-tile (A phase), written bf16 to one DRAM tensor G with
    row pitch Q=2176 (2049 band cols + 127 zero pad).
  - The shifted+TRANSPOSED band btT[jp,i] for each j-tile comes back in a
    single xbar DMA: source AP [[Q-1, iw],[1,128]] reads G along the
    diagonal; transpose=True lands it j-major.  Out-of-band reads hit the
    zero pad of the neighbouring row.
  - Band add: PE accumulation (identity @ btT into the qk PSUM group) for
    even j-tiles, DVE tensor add for odd ones — balances both engines.
  - P = exp(Lt) via ScalarE, bf16, straight into sb_Pt[j-tile][i].
  - attn@v per 512-i-chunk: po[65,512] += vba[j-tile]^T @ Pt, accumulated
    over the 16 j-tiles in PSUM; vba has a ones column so row 64 is the
    softmax denominator.  Host does the final transpose + normalize.
"""

import numpy as np
import ml_dtypes

import concourse.bass as bass
import concourse.bacc as bacc
import concourse.mybir as mybir
import concourse.tile as tile
from concourse.bass_utils import run_bass_kernel_spmd
from concourse.masks import make_identity

S = 2048
D = 64
NB = 64          # pos-emb dim (basis features)
H = 8
HALF = NB // 2   # 32
BAND = 1024      # max |d| with nonzero features
Q = S + 128      # G row pitch (2049 band cols + 127 zero pad)
NT = S // 128    # 16 i/j tiles
F32 = mybir.dt.float32
BF16 = mybir.dt.bfloat16

_NC_CACHE = {}


def _basis_feature_matrix():
    """Rr[c', n] for c' in [0, Q): features of distance d = 1024 - c'.
    Matches reference._relative_basis numerics (float32)."""
    pow_rate = np.float32(np.exp(np.log((S + 1) / 2) / HALF))
    widths = np.power(pow_rate, np.arange(1, HALF + 1, dtype=np.float32),
                      dtype=np.float32)  # [32]
    d = (np.float32(BAND) - np.arange(Q, dtype=np.float32))[:, None]  # [Q,1]
    unsigned = (np.abs(d) <= widths[None, :]).astype(np.float32)      # [Q,32]
    signed = np.sign(d) * unsigned
    return np.concatenate([unsigned, signed], axis=-1)  # [Q, 64]


def _jwin(jb):
    j0 = jb * 128
    ilo = max(0, j0 - BAND)
    ihi = min(S, j0 + 128 + BAND)
    return j0, ilo, ihi


def _build_nc():
    if "nc" in _NC_CACHE:
        return _NC_CACHE["nc"]

    nc = bacc.Bacc("TRN2", target_bir_lowering=False, debug=False,
                   num_devices=H)
    d_q = nc.dram_tensor("qTb", [65, S], BF16, kind="ExternalInput")
    d_k = nc.dram_tensor("kaug", [65, S], BF16, kind="ExternalInput")
    d_w2r = nc.dram_tensor("w2r", [65, Q], BF16, kind="ExternalInput")
    d_vba = nc.dram_tensor("vba", [128 * NT * 65], BF16, kind="ExternalInput")
    d_oT = nc.dram_tensor("oT", [65, S], F32, kind="ExternalOutput")
    d_G = nc.dram_tensor("gband", [S * Q], BF16, kind="Internal")

    with tile.TileContext(nc) as tc:
        with tc.tile_pool(name="pers", bufs=1) as pers, \
             tc.tile_pool(name="gsb", bufs=3) as gsb, \
             tc.tile_pool(name="ps_wk", bufs=3, space="PSUM") as ps_wk, \
             tc.tile_pool(name="ps_tl", bufs=1, space="PSUM") as ps_tl, \
             tc.tile_pool(name="ps_av", bufs=1, space="PSUM") as ps_av:

            # ---- persistent SBUF (one DMA per tensor, spread across
            # queues so the prologue isn't serialized on one DGE) ----
            sb_w2r = pers.tile([65, Q], BF16)
            nc.sync.dma_start(out=sb_w2r[:], in_=d_w2r[:])
            sb_q = pers.tile([65, S], BF16)
            nc.scalar.dma_start(out=sb_q[:], in_=d_q[:])
            sb_k = pers.tile([65, S], BF16)
            nc.gpsimd.dma_start(out=sb_k[:], in_=d_k[:])
            sb_vb = pers.tile([128, NT, 65], BF16)
            rdv = bass.AP(tensor=d_vba, offset=0,
                          ap=[[NT * 65, 128], [1, NT * 65]])
            nc.gpsimd.dma_start(out=sb_vb[:], in_=rdv)
            sb_idb = pers.tile([128, 128], BF16)
            make_identity(nc, sb_idb[:])
            sb_Pt = pers.tile([128, NT, S], BF16)   # [jp, jb, i]
            sb_bt = pers.tile([128, NT, Q], BF16)   # [jp, jb, i - ilo]
            sb_oT = pers.tile([65, S], F32)

            def phase_A(t, eflip=[0]):
                """Band tile t in c'-space -> G rows [t*128, t*128+128)."""
                i0 = t * 128
                jlo = max(0, i0 - BAND)
                jhi = min(S, i0 + 128 + BAND)
                clo = max(0, (jlo - i0 + BAND) - 127)
                chi = min(2049, (jhi - 1) - i0 + BAND + 1)
                gt = gsb.tile([128, Q], BF16)
                nc.gpsimd.memset(gt[:, chi:Q], 0.0)
                # big 1024-chunks in ps_wk; a <=512 remainder in ps_tl so a
                # tile ties up at most 2 of the 3 shared wk slots
                cuts = list(range(clo, chi, 1024)) + [chi]
                for ci in range(len(cuts) - 1):
                    lo, hi = cuts[ci], cuts[ci + 1]
                    if hi - lo > 512:
                        pg = ps_wk.tile([128, 1024], F32, tag="wk")
                    else:
                        pg = ps_tl.tile([128, 512], F32, tag="tl")
                    for s in range(0, hi - lo, 512):
                        se = min(hi - lo, s + 512)
                        nc.tensor.matmul(
                            pg[:, s:se],
                            lhsT=sb_q[:, i0:i0 + 128],
                            rhs=sb_w2r[:, lo + s:lo + se],
                            start=True, stop=True)
                    if eflip[0] % 2 == 0:
                        nc.vector.tensor_copy(gt[:, lo:hi], pg[:, 0:hi - lo])
                    else:
                        nc.scalar.copy(out=gt[:, lo:hi], in_=pg[:, 0:hi - lo])
                    eflip[0] += 1
                wr = bass.AP(tensor=d_G, offset=i0 * Q + clo,
                             ap=[[Q, 128], [1, Q - clo]])
                nc.gpsimd.dma_start(out=wr, in_=gt[:, clo:Q])

            def read_B(jb, h):
                """Shifted+transposed band (i-half h) for j-tile jb."""
                j0, ilo, ihi = _jwin(jb)
                rlo = max(ilo, h * 1024)
                rhi = min(ihi, (h + 1) * 1024)
                if rlo >= rhi:
                    return
                rd = bass.AP(tensor=d_G, offset=rlo * (Q - 1) + j0 + BAND,
                             ap=[[Q - 1, rhi - rlo], [1, 128]])
                nc.sync.dma_start(
                    out=sb_bt[:, jb, rlo - ilo:rhi - ilo], in_=rd,
                    transpose=True)

            def phase_C(jb, h):
                """Transposed logits + band + exp for j-tile jb, i-half h."""
                j0, ilo, ihi = _jwin(jb)
                pq = ps_wk.tile([128, 1024], F32, tag="wk")
                alo = max(ilo, h * 1024)
                ahi = min(ihi, (h + 1) * 1024)
                spans = []
                for c in range(2):
                    i0 = h * 1024 + c * 512
                    # in-band sub-range of this 512-chunk
                    blo = max(alo, i0)
                    bhi = min(ahi, i0 + 512)
                    on_pe = (jb % 2 == 0) and blo < bhi
                    spans.append((blo, bhi) if on_pe else None)
                    nc.tensor.matmul(
                        pq[:, c * 512:(c + 1) * 512],
                        lhsT=sb_k[:, j0:j0 + 128],
                        rhs=sb_q[:, i0:i0 + 512],
                        start=True, stop=not on_pe)
                for sp in spans:
                    if sp is not None:
                        blo, bhi = sp
                        nc.tensor.matmul(
                            pq[:, blo - h * 1024:bhi - h * 1024],
                            lhsT=sb_idb[:],
                            rhs=sb_bt[:, jb, blo - ilo:bhi - ilo],
                            start=False, stop=True)
                if jb % 2 == 1 and alo < ahi:
                    nc.vector.tensor_tensor(
                        pq[:, alo - h * 1024:ahi - h * 1024],
                        pq[:, alo - h * 1024:ahi - h * 1024],
                        sb_bt[:, jb, alo - ilo:ahi - ilo],
                        op=mybir.AluOpType.add)
                nc.scalar.activation(
                    out=sb_Pt[:, jb, h * 1024:(h + 1) * 1024], in_=pq[:],
                    func=mybir.ActivationFunctionType.Exp)

            def phase_AV(c):
                po = ps_av.tile([65, 512], F32, tag="po")
                for jb in range(NT):
                    nc.tensor.matmul(
                        po[:],
                        lhsT=sb_vb[:, jb, :],
                        rhs=sb_Pt[:, jb, c * 512:(c + 1) * 512],
                        start=(jb == 0), stop=(jb == NT - 1))
                nc.vector.tensor_copy(sb_oT[:, c * 512:(c + 1) * 512], po[:])

            # ---- schedule ----
            # h=0 reads only need A(0..7); emit A descending so the h=0
            # sweep (descending jb) unlocks tile by tile.  A(8..15) and the
            # h=1 reads interleave with the h=0 sweep.
            for t in range(7, -1, -1):
                phase_A(t)
            for jb in range(NT - 1, -1, -1):
                read_B(jb, 0)
            for x, jb in enumerate(range(NT - 1, -1, -1)):
                phase_C(jb, 0)
                if x % 2 == 0 and x // 2 < 8:
                    phase_A(8 + x // 2)
                else:
                    read_B(x // 2, 1)
            for jb in range(8, NT):
                read_B(jb, 1)
            phase_AV(0)
            phase_AV(1)
            for jb in range(NT):
                phase_C(jb, 1)
            phase_AV(2)
            phase_AV(3)
            for c in range(4):
                nc.sync.dma_start(out=d_oT[:, c * 512:(c + 1) * 512],
                                  in_=sb_oT[:, c * 512:(c + 1) * 512])

    nc.finalize()
    _NC_CACHE["nc"] = nc
    return nc


def _host_prep(query, key, value, u, v, w):
    """Build the 8 per-core input maps from the full inputs."""
    q = np.asarray(query, np.float32)[0]   # [S,H,D]
    k = np.asarray(key, np.float32)[0]
    val = np.asarray(value, np.float32)[0]
    u = np.asarray(u, np.float32)
    v = np.asarray(v, np.float32)
    w = np.asarray(w, np.float32)
    Rr = _basis_feature_matrix()           # [Q, 64]

    in_maps = []
    for h in range(H):
        q8 = q[:, h, :] / np.float32(8.0)                  # [S,64]
        vw8 = (v[h] @ w[h]) / np.float32(8.0)              # [64]
        qTb = np.concatenate([q8.T, np.ones((1, S), np.float32)], axis=0)
        uk8 = (u[h] / np.float32(8.0)) @ k[:, h, :].T      # [S]
        kaug = np.concatenate([k[:, h, :].T, uk8[None]], axis=0)  # [65,S]
        w2r = np.concatenate([w[h] @ Rr.T, (vw8 @ Rr.T)[None]], axis=0)
        vba = np.concatenate([val[:, h, :], np.ones((S, 1), np.float32)],
                             axis=1)                       # [S,65]
        # device layout [128, NT*65]: row p holds [v|1] for s = jb*128+p
        vba_dev = vba.reshape(NT, 128, 65).transpose(1, 0, 2).reshape(128, -1)
        in_maps.append({
            "qTb": np.ascontiguousarray(qTb).astype(ml_dtypes.bfloat16),
            "kaug": np.ascontiguousarray(kaug).astype(ml_dtypes.bfloat16),
            "w2r": np.ascontiguousarray(w2r).astype(ml_dtypes.bfloat16),
            "vba": np.ascontiguousarray(vba_dev).astype(
                ml_dtypes.bfloat16).reshape(-1),
        })
    return in_maps


def kernel(query, key, value, u, v, w, _trace=False):
    nc = _build_nc()
    in_maps = _host_prep(query, key, value, u, v, w)
    res = run_bass_kernel_spmd(nc, in_maps, core_ids=list(range(H)),
                               trace=_trace)
    outs = []
    for h in range(H):
        oT = res.results[h]["oT"]                # [65, S]
        outs.append((oT[0:D] / oT[D:D + 1]).T)   # [S, D]
    full = np.stack(outs, axis=1)[None]          # [1,S,H,D]
    out = np.ascontiguousarray(full.astype(np.float32))
    if _trace:
        return out, res
    return out


if __name__ == "__main__":
    rng = np.random.default_rng(0)
    ins = {
        "query": rng.standard_normal((1, S, H, D), np.float32),
        "key": rng.standard_normal((1, S, H, D), np.float32),
        "value": rng.standard_normal((1, S, H, D), np.float32),
        "u": rng.standard_normal((H, D), np.float32),
        "v": rng.standard_normal((H, D), np.float32),
        "w": rng.standard_normal((H, D, NB), np.float32),
    }
    out = kernel(**ins)
    print("out shape:", out.shape, "finite:", np.isfinite(out).all())


# revision 31
# speedup vs baseline: 1.2653x; 1.2594x over previous
"""Enformer dot-product self-attention with central-mask relative position
bias, on 8 Trainium2 NeuronCores (one head per core, SPMD).

Math per head h (S=2048, D=64, N=64):
    basis[i,j,:] = f(d=i-j)  — indicator features, zero for |d| > 1024
    logits = (q @ k^T + (q @ w) @ basis^T + u @ k^T + (v @ w) @ basis^T) / 8
    out    = softmax(logits) @ value

Device formulation per core (v3 — transposed/j-major attention):
  - qTb [65,S] bf16: rows 0..63 = (q/8)^T, row 64 = ones.
  - kaug [65,S] bf16: rows 0..63 = k^T, row 64 = u@k^T/8 (host-computed).
    Logits are built TRANSPOSED: Lt[j,i] = kaug_tile^T @ qTb  (kaug j-tile
    stationary, qTb streaming), so softmax rows live on the free axis and
    attn@v needs no P transposes at all.
  - Band term in c'-space: T[i,c'] = qTb[:,i].w2r[:,c'], c' = j-i+1024,
    computed per i-tile (A phase), written bf16 to one DRAM tensor G with
    row pitch Q=2176 (2049 band cols + 127 zero pad).
  - The shifted+TRANSPOSED band btT[jp,i] for each j-tile comes back in a
    single xbar DMA: source AP [[Q-1, iw],[1,128]] reads G along the
    diagonal; transpose=True lands it j-major.  Out-of-band reads hit the
    zero pad of the neighbouring row.
  - Band add: PE accumulation (identity @ btT into the qk PSUM group) for
    even j-tiles, DVE tensor add for odd ones — balances both engines.
  - P = exp(Lt) via ScalarE, bf16, straight into sb_Pt[j-tile][i].
  - attn@v per 512-i-chunk: po[65,512] += vba[j-tile]^T @ Pt, accumulated
    over the 16 j-tiles in PSUM; vba has a ones column so row 64 is the
    softmax denominator.  Host does the final transpose + normalize.
"""

import numpy as np
import ml_dtypes

import concourse.bass as bass
import concourse.bacc as bacc
import concourse.mybir as mybir
import concourse.tile as tile
from concourse.bass_utils import run_bass_kernel_spmd
from concourse.masks import make_identity

S = 2048
D = 64
NB = 64          # pos-emb dim (basis features)
H = 8
HALF = NB // 2   # 32
BAND = 1024      # max |d| with nonzero features
Q = S + 128      # G row pitch (2049 band cols + 127 zero pad)
NT = S // 128    # 16 i/j tiles
F32 = mybir.dt.float32
BF16 = mybir.dt.bfloat16

_NC_CACHE = {}


def _basis_feature_matrix():
    """Rr[c', n] for c' in [0, Q): features of distance d = 1024 - c'.
    Matches reference._relative_basis numerics (float32)."""
    pow_rate = np.float32(np.exp(np.log((S + 1) / 2) / HALF))
    widths = np.power(pow_rate, np.arange(1, HALF + 1, dtype=np.float32),
                      dtype=np.float32)  # [32]
    d = (np.float32(BAND) - np.arange(Q, dtype=np.float32))[:, None]  # [Q,1]
    unsigned = (np.abs(d) <= widths[None, :]).astype(np.float32)      # [Q,32]
    signed = np.sign(d) * unsigned
    return np.concatenate([unsigned, signed], axis=-1)  # [Q, 64]


def _jwin(jb):
    j0 = jb * 128
    ilo = max(0, j0 - BAND)
    ihi = min(S, j0 + 128 + BAND)
    return j0, ilo, ihi


def _build_nc():
    if "nc" in _NC_CACHE:
        return _NC_CACHE["nc"]

    nc = bacc.Bacc("TRN2", target_bir_lowering=False, debug=False,
                   num_devices=H)
    d_q = nc.dram_tensor("qTb", [65, S], BF16, kind="ExternalInput")
    d_k = nc.dram_tensor("kaug", [65, S], BF16, kind="ExternalInput")
    d_w2r = nc.dram_tensor("w2r", [65, Q], BF16, kind="ExternalInput")
    d_vba = nc.dram_tensor("vba", [128 * NT * 65], BF16, kind="ExternalInput")
    d_oT = nc.dram_tensor("oT", [65, S], F32, kind="ExternalOutput")
    d_G = nc.dram_tensor("gband", [S * Q], BF16, kind="Internal")

    BTP = 2048   # sb_bt row pitch (group-relative positions i - ilo(4g))

    with tile.TileContext(nc) as tc:
        with tc.tile_pool(name="pers", bufs=1) as pers, \
             tc.tile_pool(name="gsb", bufs=2) as gsb, \
             tc.tile_pool(name="ps_wk", bufs=3, space="PSUM") as ps_wk, \
             tc.tile_pool(name="ps_tl", bufs=1, space="PSUM") as ps_tl, \
             tc.tile_pool(name="ps_av", bufs=1, space="PSUM") as ps_av:

            # ---- persistent SBUF (one DMA per tensor; HWDGE queues only —
            # SWDGE descriptor rings interfere with xbar transposes) ----
            sb_w2r = pers.tile([65, Q], BF16)
            nc.scalar.dma_start(out=sb_w2r[:], in_=d_w2r[:])
            sb_q = pers.tile([65, S], BF16)
            nc.scalar.dma_start(out=sb_q[:], in_=d_q[:])
            sb_k = pers.tile([65, S], BF16)
            nc.sync.dma_start(out=sb_k[:], in_=d_k[:])
            sb_vb = pers.tile([128, NT, 65], BF16)
            rdv = bass.AP(tensor=d_vba, offset=0,
                          ap=[[NT * 65, 128], [1, NT * 65]])
            nc.sync.dma_start(out=sb_vb[:], in_=rdv)
            sb_idb = pers.tile([128, 128], BF16)
            make_identity(nc, sb_idb[:])
            sb_Pt = pers.tile([128, NT, S], BF16)    # [jp, jb, i]
            sb_bt = pers.tile([128, 4, 4, BTP], BF16)  # [jp, g, b, i-ilo(4g)]
            sb_oT = pers.tile([65, S], F32)

            def phase_A(t, sweep=False, eflip=[0]):
                """Band tile t in c'-space -> G rows [t*128, t*128+128)."""
                i0 = t * 128
                jlo = max(0, i0 - BAND)
                jhi = min(S, i0 + 128 + BAND)
                clo = max(0, (jlo - i0 + BAND) - 127)
                chi = min(2049, (jhi - 1) - i0 + BAND + 1)
                gt = gsb.tile([128, Q], BF16)
                nc.gpsimd.memset(gt[:, chi:Q], 0.0)
                # big 1024-chunks in ps_wk; a <=512 remainder in ps_tl so a
                # tile ties up at most 2 of the 3 shared wk slots
                cuts = list(range(clo, chi, 1024)) + [chi]
                for ci in range(len(cuts) - 1):
                    lo, hi = cuts[ci], cuts[ci + 1]
                    if hi - lo > 512:
                        pg = ps_wk.tile([128, 1024], F32, tag="wk")
                    else:
                        pg = ps_tl.tile([128, 512], F32, tag="tl")
                    for s in range(0, hi - lo, 512):
                        se = min(hi - lo, s + 512)
                        nc.tensor.matmul(
                            pg[:, s:se],
                            lhsT=sb_q[:, i0:i0 + 128],
                            rhs=sb_w2r[:, lo + s:lo + se],
                            start=True, stop=True)
                    # during the C' sweep ScalarE is exp-bound: evac on DVE
                    if sweep or eflip[0] % 2 == 0:
                        nc.vector.tensor_copy(gt[:, lo:hi], pg[:, 0:hi - lo])
                    else:
                        nc.scalar.copy(out=gt[:, lo:hi], in_=pg[:, 0:hi - lo])
                    eflip[0] += 1
                wr = bass.AP(tensor=d_G, offset=i0 * Q + clo,
                             ap=[[Q, 128], [1, Q - clo]])
                nc.sync.dma_start(out=wr, in_=gt[:, clo:Q])

            def read_B4(g, h):
                """Shifted+transposed band (i-half h) for j-tiles 4g..4g+3,
                one batched xbar DMA.  Within a group the per-tile window
                starts differ by a constant delta (0 or 128); storing rows
                at group-relative positions BTOFF + i - ilo(4g) makes the
                batched dest a natural [part, b, i] slice of sb_bt."""
                j0 = 4 * g * 128
                wins = [_jwin(4 * g + b) for b in range(4)]
                rlo = [max(w[1], h * 1024) for w in wins]
                rhi = [min(w[2], (h + 1) * 1024) for w in wins]
                rmin, rmax = min(rlo), max(rhi)
                if rmin >= rmax:
                    return
                ilo0 = wins[0][1]
                rd = bass.AP(tensor=d_G, offset=rmin * (Q - 1) + j0 + BAND,
                             ap=[[Q - 1, rmax - rmin], [1, 512]])
                ypos = rmin - ilo0
                nc.sync.dma_start(
                    out=sb_bt[:, g, :, ypos:ypos + (rmax - rmin)], in_=rd,
                    transpose=True)

            def phase_C(jb, h):
                """Transposed logits + band + exp for j-tile jb, i-half h."""
                j0, ilo, ihi = _jwin(jb)
                pq = ps_wk.tile([128, 1024], F32, tag="wk")
                alo = max(ilo, h * 1024)
                ahi = min(ihi, (h + 1) * 1024)
                spans = []
                for c in range(2):
                    i0 = h * 1024 + c * 512
                    # in-band sub-range of this 512-chunk
                    blo = max(alo, i0)
                    bhi = min(ahi, i0 + 512)
                    on_pe = (jb % 2 == 0) and blo < bhi
                    spans.append((blo, bhi) if on_pe else None)
                    nc.tensor.matmul(
                        pq[:, c * 512:(c + 1) * 512],
                        lhsT=sb_k[:, j0:j0 + 128],
                        rhs=sb_q[:, i0:i0 + 512],
                        start=True, stop=not on_pe)
                g, b = jb // 4, jb % 4
                ilo0 = _jwin(4 * g)[1]
                for sp in spans:
                    if sp is not None:
                        blo, bhi = sp
                        nc.tensor.matmul(
                            pq[:, blo - h * 1024:bhi - h * 1024],
                            lhsT=sb_idb[:],
                            rhs=sb_bt[:, g, b, blo - ilo0:bhi - ilo0],
                            start=False, stop=True)
                if jb % 2 == 1 and alo < ahi:
                    nc.vector.tensor_tensor(
                        pq[:, alo - h * 1024:ahi - h * 1024],
                        pq[:, alo - h * 1024:ahi - h * 1024],
                        sb_bt[:, g, b, alo - ilo0:ahi - ilo0],
                        op=mybir.AluOpType.add)
                nc.scalar.activation(
                    out=sb_Pt[:, jb, h * 1024:(h + 1) * 1024], in_=pq[:],
                    func=mybir.ActivationFunctionType.Exp)

            def phase_AV(c):
                po = ps_av.tile([65, 512], F32, tag="po")
                for jb in range(NT):
                    nc.tensor.matmul(
                        po[:],
                        lhsT=sb_vb[:, jb, :],
                        rhs=sb_Pt[:, jb, c * 512:(c + 1) * 512],
                        start=(jb == 0), stop=(jb == NT - 1))
                nc.vector.tensor_copy(sb_oT[:, c * 512:(c + 1) * 512], po[:])

            # ---- schedule ----
            # h=0 group reads need: g3 -> A(4..7); g0..g2 -> A(0..7).
            # Emit A descending so group reads unlock progressively; the
            # h=0 sweep runs jb descending.  A(8..15) and the h=1 reads
            # interleave with the h=0 sweep.
            for t in range(7, 3, -1):
                phase_A(t)
            read_B4(3, 0)
            for t in range(3, -1, -1):
                phase_A(t)
            read_B4(2, 0)
            read_B4(1, 0)
            read_B4(0, 0)
            for x, jb in enumerate(range(NT - 1, -1, -1)):
                phase_C(jb, 0)
                if x % 2 == 0 and x // 2 < 8:
                    phase_A(8 + x // 2, sweep=True)
                elif x == 9:
                    read_B4(0, 1)   # union rows [1024,1536) -> needs A(11)
            # g1..g3 h=1 unions all reach row 2048 -> must follow A(15)
            read_B4(1, 1)
            read_B4(2, 1)
            read_B4(3, 1)
            phase_AV(0)
            phase_AV(1)
            for jb in range(NT):
                phase_C(jb, 1)
            phase_AV(2)
            phase_AV(3)
            for c in range(4):
                nc.sync.dma_start(out=d_oT[:, c * 512:(c + 1) * 512],
                                  in_=sb_oT[:, c * 512:(c + 1) * 512])

    nc.finalize()
    _NC_CACHE["nc"] = nc
    return nc


def _host_prep(query, key, value, u, v, w):
    """Build the 8 per-core input maps from the full inputs."""
    q = np.asarray(query, np.float32)[0]   # [S,H,D]
    k = np.asarray(key, np.float32)[0]
    val = np.asarray(value, np.float32)[0]
    u = np.asarray(u, np.float32)
    v = np.asarray(v, np.float32)
    w = np.asarray(w, np.float32)
    Rr = _basis_feature_matrix()           # [Q, 64]

    in_maps = []
    for h in range(H):
        q8 = q[:, h, :] / np.float32(8.0)                  # [S,64]
        vw8 = (v[h] @ w[h]) / np.float32(8.0)              # [64]
        qTb = np.concatenate([q8.T, np.ones((1, S), np.float32)], axis=0)
        uk8 = (u[h] / np.float32(8.0)) @ k[:, h, :].T      # [S]
        kaug = np.concatenate([k[:, h, :].T, uk8[None]], axis=0)  # [65,S]
        w2r = np.concatenate([w[h] @ Rr.T, (vw8 @ Rr.T)[None]], axis=0)
        vba = np.concatenate([val[:, h, :], np.ones((S, 1), np.float32)],
                             axis=1)                       # [S,65]
        # device layout [128, NT*65]: row p holds [v|1] for s = jb*128+p
        vba_dev = vba.reshape(NT, 128, 65).transpose(1, 0, 2).reshape(128, -1)
        in_maps.append({
            "qTb": np.ascontiguousarray(qTb).astype(ml_dtypes.bfloat16),
            "kaug": np.ascontiguousarray(kaug).astype(ml_dtypes.bfloat16),
            "w2r": np.ascontiguousarray(w2r).astype(ml_dtypes.bfloat16),
            "vba": np.ascontiguousarray(vba_dev).astype(
                ml_dtypes.bfloat16).reshape(-1),
        })
    return in_maps


def kernel(query, key, value, u, v, w, _trace=False):
    nc = _build_nc()
    in_maps = _host_prep(query, key, value, u, v, w)
    res = run_bass_kernel_spmd(nc, in_maps, core_ids=list(range(H)),
                               trace=_trace)
    outs = []
    for h in range(H):
        oT = res.results[h]["oT"]                # [65, S]
        outs.append((oT[0:D] / oT[D:D + 1]).T)   # [S, D]
    full = np.stack(outs, axis=1)[None]          # [1,S,H,D]
    out = np.ascontiguousarray(full.astype(np.float32))
    if _trace:
        return out, res
    return out


if __name__ == "__main__":
    rng = np.random.default_rng(0)
    ins = {
        "query": rng.standard_normal((1, S, H, D), np.float32),
        "key": rng.standard_normal((1, S, H, D), np.float32),
        "value": rng.standard_normal((1, S, H, D), np.float32),
        "u": rng.standard_normal((H, D), np.float32),
        "v": rng.standard_normal((H, D), np.float32),
        "w": rng.standard_normal((H, D, NB), np.float32),
    }
    out = kernel(**ins)
    print("out shape:", out.shape, "finite:", np.isfinite(out).all())
